# revision 5
# baseline (speedup 1.0000x reference)
"""BrainGAT (3-layer GAT + mean-pool + projection) on 8 Trainium2 NeuronCores.

Strategy (dst-sharded message passing):
  - Nodes are sharded by destination across the 8 cores (4096 nodes each, whole
    graphs).  Edges (incl. self-loops) are sorted by dst on the host and padded
    into 128-edge tiles per 128-node dst block.
  - Per layer, each core gathers the per-src feature rows (dma_gather), weights
    them with the edge attention p = exp(leaky_relu(als[src]+ald[dst])) and
    scatter-adds into PSUM with a host-built 0/1 selection matrix S via the
    tensor engine:  out[d,:] = S^T @ (p * xw[src]);  denom[d,h] = S^T @ p.
    Softmax max-subtraction is skipped (mathematically identical, logits are
    O(1) so exp never overflows).
  - The aggregation for layer 1 runs in input space (x is only 4 wide) and W1
    is applied after aggregation (linearity of the attention-weighted sum).
  - Between layers each core computes the next layer's gather table rows
    (xw_{l+1}, als_{l+1}, ald_{l+1}) for its own 4096 nodes and the tables are
    AllGathered so every core can gather arbitrary src rows.
  - Final mean-pool is per-graph-local (graphs never cross cores); each core
    emits pooled @ Wp + bp for its 8 graphs, host concatenates.
"""

import sys
import numpy as np

for _p in ("/opt/trn_rl_repo",):
    if _p not in sys.path:
        sys.path.insert(0, _p)

import ml_dtypes
import concourse.bass as bass
import concourse.bacc as bacc
import concourse.mybir as mybir
import concourse.tile as tile
from concourse.bass_utils import run_bass_kernel_spmd
from concourse.masks import make_identity

dt = mybir.dt
F32, BF16, I16 = dt.float32, dt.bfloat16, dt.int16
BF16_NP = ml_dtypes.bfloat16

N = 32768
G = 64
IN = 4
H = 8
C = 64
OUT = 384
NEG = 0.2
NCORES = 8
NPC = N // NCORES          # nodes per core
NBLK = NPC // 128          # dst blocks per core
P = 128

BENCH_REPS = 0             # test.py sets >0 to time repeated device calls
LAST_BENCH_NS = None

_cache = {}


def _build(k_list, nonzero_bias):
    K_tot = sum(k_list)
    offs = np.concatenate([[0], np.cumsum(k_list)]).astype(int)

    nc = bacc.Bacc("TRN2", target_bir_lowering=False, debug=False,
                   num_devices=NCORES)

    # ---- I/O --------------------------------------------------------------
    x_in = nc.dram_tensor("x", [N, IN], F32, kind="ExternalInput")
    xT_in = nc.dram_tensor("xT", [IN, N], F32, kind="ExternalInput")
    wsd1_in = nc.dram_tensor("wsd1", [IN, 2 * H], F32, kind="ExternalInput")
    w1b_in = nc.dram_tensor("w1b", [H * IN, H * C], BF16, kind="ExternalInput")
    w2c_in = nc.dram_tensor("w2c", [128, 4 * 528], BF16, kind="ExternalInput")
    w3c_in = nc.dram_tensor("w3c", [128, 4 * 66], BF16, kind="ExternalInput")
    wp_in = nc.dram_tensor("wp", [C, OUT], F32, kind="ExternalInput")
    bp_in = nc.dram_tensor("bp", [8, OUT], F32, kind="ExternalInput")
    poh_in = nc.dram_tensor("poh", [128, NBLK * 8], F32, kind="ExternalInput")
    isrc_in = nc.dram_tensor("isrc", [16, K_tot * 8], I16, kind="ExternalInput")
    idstg_in = nc.dram_tensor("idstg", [16, K_tot * 8], I16, kind="ExternalInput")
    idstl_in = nc.dram_tensor("idstl", [16, K_tot * 8], I16, kind="ExternalInput")
    S_in = nc.dram_tensor("S", [128, K_tot * 128], BF16, kind="ExternalInput")
    out_ext = nc.dram_tensor("out", [8, OUT], F32, kind="ExternalOutput")
    if nonzero_bias:
        b1_in = nc.dram_tensor("b1r", [128, H * C], F32, kind="ExternalInput")
        b2_in = nc.dram_tensor("b2r", [128, H * C], F32, kind="ExternalInput")
        b3_in = nc.dram_tensor("b3r", [128, C], F32, kind="ExternalInput")

    # ---- internal DRAM tables --------------------------------------------
    # aux1: [x(4) | als1(8) | ald1(8) | pad] f32, 256B rows, replicated per core
    aux1 = nc.dram_tensor("aux1", [N, 64], F32)
    # main2: [xw2 bf16 512 (256w) | als2 f32 8 | pad] 1280B rows
    main2_own = nc.dram_tensor("main2_own", [NPC, 320], F32)
    main2 = nc.dram_tensor("main2", [N, 320], F32, addr_space="Shared")
    # auxd2: [ald2 f32 8 | pad] 256B rows, core-local (indexed by local dst)
    auxd2 = nc.dram_tensor("auxd2", [NPC, 64], F32)
    # main3: [xw3 bf16 64 (32w) | als3 f32 | ald3 f32 | pad] 256B rows
    main3_own = nc.dram_tensor("main3_own", [NPC, 64], F32)
    main3 = nc.dram_tensor("main3", [N, 64], F32, addr_space="Shared")

    RG = [list(range(NCORES))]

    with tile.TileContext(nc) as tc:
        with tc.tile_pool(name="const", bufs=1) as cst, \
             tc.tile_pool(name="gbuf", bufs=2) as sbg, \
             tc.tile_pool(name="small", bufs=2) as sbm, \
             tc.tile_pool(name="psA", bufs=2, space="PSUM") as psA, \
             tc.tile_pool(name="psD", bufs=1, space="PSUM") as psD, \
             tc.tile_pool(name="psT", bufs=1, space="PSUM") as psT, \
             tc.tile_pool(name="psH", bufs=1, space="PSUM") as psH, \
             tc.tile_pool(name="psX", bufs=1, space="PSUM") as psX, \
             tc.tile_pool(name="psS", bufs=1, space="PSUM") as psS, \
             tc.tile_pool(name="psP", bufs=1, space="PSUM") as psP:

            # ---- constants into SBUF -------------------------------------
            ident = cst.tile([128, 128], F32)
            make_identity(nc, ident[:])
            identb = cst.tile([128, 128], BF16)
            nc.vector.tensor_copy(out=identb[:], in_=ident[:])
            wsd1 = cst.tile([IN, 2 * H], F32)
            nc.sync.dma_start(out=wsd1[:], in_=wsd1_in[:, :])
            w1b = cst.tile([H * IN, H * C], BF16)
            nc.sync.dma_start(out=w1b[:], in_=w1b_in[:, :])
            w2c = cst.tile([128, 4 * 528], BF16)
            nc.sync.dma_start(out=w2c[:], in_=w2c_in[:, :])
            w3c = cst.tile([128, 4 * 66], BF16)
            nc.sync.dma_start(out=w3c[:], in_=w3c_in[:, :])
            wp = cst.tile([C, OUT], F32)
            nc.sync.dma_start(out=wp[:], in_=wp_in[:, :])
            bp = cst.tile([8, OUT], F32)
            nc.sync.dma_start(out=bp[:], in_=bp_in[:, :])
            poh = cst.tile([128, NBLK * 8], F32)
            nc.sync.dma_start(out=poh[:], in_=poh_in[:, :])
            if nonzero_bias:
                b1r = cst.tile([128, H * C], F32)
                nc.sync.dma_start(out=b1r[:], in_=b1_in[:, :])
                b2r = cst.tile([128, H * C], F32)
                nc.sync.dma_start(out=b2r[:], in_=b2_in[:, :])
                b3r = cst.tile([128, C], F32)
                nc.sync.dma_start(out=b3r[:], in_=b3_in[:, :])

            isrc = cst.tile([128, K_tot * 8], I16)
            idstg = cst.tile([128, K_tot * 8], I16)
            idstl = cst.tile([128, K_tot * 8], I16)
            for g in range(8):
                nc.sync.dma_start(out=isrc[g * 16:(g + 1) * 16, :], in_=isrc_in[:, :])
                nc.sync.dma_start(out=idstg[g * 16:(g + 1) * 16, :], in_=idstg_in[:, :])
                nc.sync.dma_start(out=idstl[g * 16:(g + 1) * 16, :], in_=idstl_in[:, :])

            # ---- stage 0: aux1 = [x | als1 | ald1] for ALL N (replicated) -
            nc.sync.dma_start(out=aux1[:, 0:IN], in_=x_in[:, :])
            for t in range(N // 128):
                if t % 16 == 0:
                    xtc = sbm.tile([IN, 16 * 128], F32, tag="xtc", bufs=2)
                    nc.sync.dma_start(out=xtc[:], in_=xT_in[:, t * 128:(t + 16) * 128])
                ps0 = psA.tile([128, 2 * H], F32, tag="agg")
                nc.tensor.matmul(out=ps0[:], lhsT=xtc[:, (t % 16) * 128:(t % 16 + 1) * 128],
                                 rhs=wsd1[:], start=True, stop=True)
                st0 = sbm.tile([128, 2 * H], F32, tag="st0", bufs=3)
                nc.vector.tensor_copy(out=st0[:], in_=ps0[:])
                nc.sync.dma_start(out=aux1[t * 128:(t + 1) * 128, IN:IN + 2 * H], in_=st0[:])

            # ---- layer template ------------------------------------------
            def layer(l):
                if l == 1:
                    main_tbl, Rw = aux1, 64
                    dst_tbl, Dw, idx_d = aux1, 64, idstg
                    a_s0, d_s0, Hl, Wl = IN, IN + H, H, H * IN
                elif l == 2:
                    main_tbl, Rw = main2, 320
                    dst_tbl, Dw, idx_d = auxd2, 64, idstl
                    a_s0, d_s0, Hl, Wl = 256, 0, H, H * C
                else:
                    main_tbl, Rw = main3, 64
                    dst_tbl, Dw, idx_d = main3, 64, idstg
                    a_s0, d_s0, Hl, Wl = 32, 33, 1, C

                for b in range(NBLK):
                    k = k_list[b]
                    off = int(offs[b])
                    nidx = k * 128
                    Gm = sbg.tile([128, k * Rw], F32, tag="gmain")
                    nc.gpsimd.dma_gather(
                        out_ap=Gm[:].rearrange("p (k r) -> p k r", r=Rw),
                        in_ap=main_tbl[:, :],
                        idxs_ap=isrc[:, off * 8:(off + k) * 8],
                        num_idxs=nidx, num_idxs_reg=nidx, elem_size=Rw,
                        single_packet=False)
                    Gd = sbg.tile([128, k * Dw], F32, tag="gdst")
                    nc.gpsimd.dma_gather(
                        out_ap=Gd[:].rearrange("p (k r) -> p k r", r=Dw),
                        in_ap=dst_tbl[:, :],
                        idxs_ap=idx_d[:, off * 8:(off + k) * 8],
                        num_idxs=nidx, num_idxs_reg=nidx, elem_size=Dw,
                        single_packet=False)
                    S = sbg.tile([128, k * 128], BF16, tag="S")
                    nc.sync.dma_start(out=S[:], in_=S_in[:, off * 128:(off + k) * 128])

                    # edge logits -> p
                    ppre = sbm.tile([128, k * Hl], F32, tag="ppre")
                    nc.vector.tensor_tensor(
                        out=ppre[:].rearrange("p (k h) -> p k h", h=Hl),
                        in0=Gm[:].rearrange("p (k r) -> p k r", r=Rw)[:, :, a_s0:a_s0 + Hl],
                        in1=Gd[:].rearrange("p (k r) -> p k r", r=Dw)[:, :, d_s0:d_s0 + Hl],
                        op=mybir.AluOpType.add)
                    psc = sbm.tile([128, k * Hl], F32, tag="psc")
                    nc.vector.tensor_scalar(out=psc[:], in0=ppre[:], scalar1=NEG,
                                            scalar2=None, op0=mybir.AluOpType.mult)
                    plog = sbm.tile([128, k * Hl], F32, tag="plog")
                    nc.vector.tensor_tensor(out=plog[:], in0=ppre[:], in1=psc[:],
                                            op=mybir.AluOpType.max)
                    pf = sbm.tile([128, k * Hl], F32, tag="pf")
                    nc.scalar.activation(out=pf[:], in_=plog[:],
                                         func=mybir.ActivationFunctionType.Exp)
                    pb = sbm.tile([128, k * Hl], BF16, tag="pb")
                    nc.vector.tensor_copy(out=pb[:], in_=pf[:])

                    agg = psA.tile([128, Wl], F32, tag="agg")
                    den = psD.tile([128, Hl], F32, tag="den")
                    for j in range(k):
                        Gw = sbm.tile([128, Wl], BF16, tag="gw", bufs=3)
                        if l == 1:
                            nc.vector.tensor_tensor(
                                out=Gw[:].rearrange("p (h c) -> p h c", c=IN),
                                in0=Gm[:, j * Rw:j * Rw + IN].unsqueeze(1).to_broadcast([128, H, IN]),
                                in1=pf[:, j * H:(j + 1) * H].unsqueeze(2).to_broadcast([128, H, IN]),
                                op=mybir.AluOpType.mult)
                        elif l == 2:
                            nc.vector.tensor_tensor(
                                out=Gw[:].rearrange("p (h c) -> p h c", c=C),
                                in0=Gm[:, j * Rw:j * Rw + 256].bitcast(BF16).rearrange(
                                    "p (h c) -> p h c", c=C),
                                in1=pf[:, j * H:(j + 1) * H].unsqueeze(2).to_broadcast([128, H, C]),
                                op=mybir.AluOpType.mult)
                        else:
                            nc.vector.tensor_tensor(
                                out=Gw[:],
                                in0=Gm[:, j * Rw:j * Rw + 32].bitcast(BF16),
                                in1=pf[:, j:j + 1].to_broadcast([128, C]),
                                op=mybir.AluOpType.mult)
                        Sj = S[:, j * 128:(j + 1) * 128]
                        nc.tensor.matmul(out=agg[:], lhsT=Sj, rhs=Gw[:],
                                         start=(j == 0), stop=(j == k - 1))
                        nc.tensor.matmul(out=den[:], lhsT=Sj,
                                         rhs=pb[:, j * Hl:(j + 1) * Hl],
                                         start=(j == 0), stop=(j == k - 1))

                    rden = sbm.tile([128, Hl], F32, tag="rden")
                    nc.vector.reciprocal(out=rden[:], in_=den[:])

                    if l == 1:
                        A1 = sbm.tile([128, Wl], BF16, tag="a1")
                        nc.vector.tensor_tensor(
                            out=A1[:].rearrange("p (h c) -> p h c", c=IN),
                            in0=agg[:].rearrange("p (h c) -> p h c", c=IN),
                            in1=rden[:].unsqueeze(2).to_broadcast([128, H, IN]),
                            op=mybir.AluOpType.mult)
                        pT = psT.tile([Wl, 128], BF16, tag="tr")
                        nc.tensor.transpose(out=pT[:], in_=A1[:], identity=identb[:])
                        A1T = sbm.tile([Wl, 128], BF16, tag="a1t")
                        nc.vector.tensor_copy(out=A1T[:], in_=pT[:])
                        hps = psH.tile([128, H * C], F32, tag="h")
                        nc.tensor.matmul(out=hps[:], lhsT=A1T[:], rhs=w1b[:],
                                         start=True, stop=True)
                        u_src = hps
                    else:
                        u = sbm.tile([128, Wl], F32, tag="u")
                        if l == 2:
                            nc.vector.tensor_tensor(
                                out=u[:].rearrange("p (h c) -> p h c", c=C),
                                in0=agg[:].rearrange("p (h c) -> p h c", c=C),
                                in1=rden[:].unsqueeze(2).to_broadcast([128, H, C]),
                                op=mybir.AluOpType.mult)
                        else:
                            nc.vector.tensor_tensor(
                                out=u[:], in0=agg[:],
                                in1=rden[:].to_broadcast([128, C]),
                                op=mybir.AluOpType.mult)
                        u_src = u

                    if nonzero_bias:
                        br = b1r if l == 1 else (b2r if l == 2 else b3r)
                        ub = sbm.tile([128, Wl], F32, tag="ub")
                        nc.vector.tensor_tensor(out=ub[:], in0=u_src[:],
                                                in1=br[:, 0:Wl],
                                                op=mybir.AluOpType.add)
                        u_src = ub

                    HWl = H * C if l == 1 else Wl
                    m_ = sbm.tile([128, HWl], F32, tag="m")
                    nc.vector.tensor_scalar(out=m_[:], in0=u_src[:], scalar1=0.0,
                                            scalar2=None, op0=mybir.AluOpType.min)
                    e_ = sbm.tile([128, HWl], F32, tag="e")
                    nc.scalar.activation(out=e_[:], in_=m_[:],
                                         func=mybir.ActivationFunctionType.Exp)
                    r_ = sbm.tile([128, HWl], F32, tag="r")
                    nc.vector.tensor_scalar(out=r_[:], in0=u_src[:], scalar1=0.0,
                                            scalar2=-1.0, op0=mybir.AluOpType.max,
                                            op1=mybir.AluOpType.add)
                    hdt = F32 if l == 3 else BF16
                    h = sbm.tile([128, HWl], hdt, tag="hout")
                    nc.vector.tensor_tensor(out=h[:], in0=r_[:], in1=e_[:],
                                            op=mybir.AluOpType.add)

                    if l == 1 or l == 2:
                        hTs = []
                        for ch in range(4):
                            tps = psT.tile([128, 128], BF16, tag="tr")
                            nc.tensor.transpose(out=tps[:],
                                                in_=h[:, ch * 128:(ch + 1) * 128],
                                                identity=identb[:])
                            hT = sbm.tile([128, 128], BF16, tag=f"ht{ch}")
                            nc.vector.tensor_copy(out=hT[:], in_=tps[:])
                            hTs.append(hT)
                        if l == 1:
                            xw = psX.tile([128, 512], F32, tag="xw")
                            als = psS.tile([128, 16], F32, tag="als")
                            for ch in range(4):
                                nc.tensor.matmul(out=xw[:], lhsT=hTs[ch][:],
                                                 rhs=w2c[:, ch * 528:ch * 528 + 512],
                                                 start=(ch == 0), stop=(ch == 3))
                            for ch in range(4):
                                nc.tensor.matmul(out=als[:], lhsT=hTs[ch][:],
                                                 rhs=w2c[:, ch * 528 + 512:(ch + 1) * 528],
                                                 start=(ch == 0), stop=(ch == 3))
                            stg = sbm.tile([128, 320], F32, tag="stg")
                            nc.vector.tensor_copy(out=stg[:, 0:256].bitcast(BF16), in_=xw[:])
                            nc.vector.tensor_copy(out=stg[:, 256:264], in_=als[:, 0:8])
                            stga = sbm.tile([128, 64], F32, tag="stga")
                            nc.vector.tensor_copy(out=stga[:, 0:8], in_=als[:, 8:16])
                            nc.sync.dma_start(out=main2_own[b * 128:(b + 1) * 128, :], in_=stg[:])
                            nc.sync.dma_start(out=auxd2[b * 128:(b + 1) * 128, :], in_=stga[:])
                        else:
                            x3 = psX.tile([128, 66], F32, tag="xw")
                            for ch in range(4):
                                nc.tensor.matmul(out=x3[:], lhsT=hTs[ch][:],
                                                 rhs=w3c[:, ch * 66:(ch + 1) * 66],
                                                 start=(ch == 0), stop=(ch == 3))
                            stg3 = sbm.tile([128, 64], F32, tag="stg")
                            nc.vector.tensor_copy(out=stg3[:, 0:32].bitcast(BF16), in_=x3[:, 0:64])
                            nc.vector.tensor_copy(out=stg3[:, 32:34], in_=x3[:, 64:66])
                            nc.sync.dma_start(out=main3_own[b * 128:(b + 1) * 128, :], in_=stg3[:])
                    else:
                        nc.tensor.matmul(out=pool_ps[:],
                                         lhsT=poh[:, b * 8:(b + 1) * 8],
                                         rhs=h[:], start=(b == 0), stop=(b == NBLK - 1))

            layer(1)
            nc.gpsimd.collective_compute(
                "AllGather", mybir.AluOpType.bypass, replica_groups=RG,
                ins=[main2_own[:, :]], outs=[main2[:, :]])
            layer(2)
            nc.gpsimd.collective_compute(
                "AllGather", mybir.AluOpType.bypass, replica_groups=RG,
                ins=[main3_own[:, :]], outs=[main3[:, :]])
            pool_ps = psP.tile([8, C], F32, tag="pool")
            layer(3)

            pooled = sbm.tile([8, C], F32, tag="pooled")
            nc.vector.tensor_scalar(out=pooled[:], in0=pool_ps[:],
                                    scalar1=1.0 / 512.0, scalar2=None,
                                    op0=mybir.AluOpType.mult)
            pTps = psT.tile([C, 8], F32, tag="tr")
            nc.tensor.transpose(out=pTps[:], in_=pooled[:], identity=ident[0:8, 0:8])
            pT = sbm.tile([C, 8], F32, tag="pT")
            nc.vector.tensor_copy(out=pT[:], in_=pTps[:])
            ops = psX.tile([8, OUT], F32, tag="xw")
            nc.tensor.matmul(out=ops[:], lhsT=pT[:], rhs=wp[:], start=True, stop=True)
            ores = sbm.tile([8, OUT], F32, tag="ores")
            nc.vector.tensor_tensor(out=ores[:], in0=ops[:], in1=bp[:],
                                    op=mybir.AluOpType.add)
            nc.sync.dma_start(out=out_ext[:, :], in_=ores[:])

    nc.compile()
    return nc


def _wrap16(a):
    # flat index position i -> [i % 16, i // 16] int16
    n = a.shape[-1]
    return np.ascontiguousarray(a.reshape(*a.shape[:-1], n // 16, 16).swapaxes(-1, -2))


def _prep_edges(edge_index):
    src = np.concatenate([np.asarray(edge_index[0], np.int64), np.arange(N, dtype=np.int64)])
    dst = np.concatenate([np.asarray(edge_index[1], np.int64), np.arange(N, dtype=np.int64)])
    order = np.argsort(dst, kind="stable")
    src_s, dst_s = src[order], dst[order]
    blk = dst_s // 128
    counts = np.bincount(blk, minlength=NCORES * NBLK)
    cgrid = counts.reshape(NCORES, NBLK)
    k_list = [max(1, int(np.ceil(cgrid[:, b].max() / 128.0))) for b in range(NBLK)]
    offs = np.concatenate([[0], np.cumsum(k_list)]).astype(int)
    K_tot = int(offs[-1])
    bstart = np.concatenate([[0], np.cumsum(counts)]).astype(int)

    isrc = np.zeros((NCORES, K_tot * 128), np.int16)
    idg = np.zeros((NCORES, K_tot * 128), np.int16)
    idl = np.zeros((NCORES, K_tot * 128), np.int16)
    S = np.zeros((NCORES, 128, K_tot * 128), BF16_NP)
    for c in range(NCORES):
        for b in range(NBLK):
            g = c * NBLK + b
            s0, s1 = bstart[g], bstart[g + 1]
            ne = s1 - s0
            off = int(offs[b])
            pos = slice(off * 128, off * 128 + ne)
            isrc[c, pos] = src_s[s0:s1]
            idg[c, pos] = dst_s[s0:s1]
            idl[c, pos] = dst_s[s0:s1] - c * NPC
            i = np.arange(ne)
            dloc = dst_s[s0:s1] - g * 128
            S[c, i % 128, (off + i // 128) * 128 + dloc] = 1.0
            # padding edges: idx stay 0 (valid row), S rows stay all-zero
    return k_list, _wrap16(isrc), _wrap16(idg), _wrap16(idl), S


def _prep_weights(W1, a1s, a1d, W2, a2s, a2d, W3, a3s, a3d, Wp, bp):
    f8 = np.float64

    def fuse(W, a):  # [D, Hl*C] x [Hl, C] -> [D, Hl]
        Hl = a.shape[0]
        Wr = W.astype(f8).reshape(W.shape[0], Hl, C)
        return np.einsum("ihc,hc->ih", Wr, a.astype(f8)).astype(np.float32)

    wsd1 = np.concatenate([fuse(W1, a1s), fuse(W1, a1d)], axis=1)  # [4, 16]
    w1b = np.zeros((H * IN, H * C), np.float32)
    for h in range(H):
        w1b[h * IN:(h + 1) * IN, h * C:(h + 1) * C] = W1[:, h * C:(h + 1) * C]
    w2cat = np.concatenate([W2, fuse(W2, a2s), fuse(W2, a2d)], axis=1)  # [512, 528]
    w3cat = np.concatenate([W3, fuse(W3, a3s), fuse(W3, a3d)], axis=1)  # [512, 66]

    def chunks(Wc):  # [512, M] -> [128, 4*M]
        return np.concatenate([Wc[ch * 128:(ch + 1) * 128, :] for ch in range(4)],
                              axis=1)

    poh = np.zeros((128, NBLK * 8), np.float32)
    for b in range(NBLK):
        poh[:, b * 8 + b // 4] = 1.0
    return dict(
        wsd1=wsd1.astype(np.float32),
        w1b=w1b.astype(BF16_NP),
        w2c=chunks(w2cat).astype(BF16_NP),
        w3c=chunks(w3cat).astype(BF16_NP),
        wp=np.asarray(Wp, np.float32),
        bp=np.tile(np.asarray(bp, np.float32)[None, :], (8, 1)),
        poh=poh,
    )


def kernel(x, edge_index, edge_attr, batch,
           W1, a1s, a1d, b1, W2, a2s, a2d, b2, W3, a3s, a3d, b3, Wp, bp):
    global LAST_BENCH_NS
    x = np.asarray(x, np.float32)
    k_list, isrc, idg, idl, S = _prep_edges(np.asarray(edge_index))
    wdict = _prep_weights(np.asarray(W1), np.asarray(a1s), np.asarray(a1d),
                          np.asarray(W2), np.asarray(a2s), np.asarray(a2d),
                          np.asarray(W3), np.asarray(a3s), np.asarray(a3d),
                          np.asarray(Wp), np.asarray(bp))
    nonzero_bias = any(np.any(np.asarray(b) != 0) for b in (b1, b2, b3))

    key = (tuple(k_list), nonzero_bias)
    if key not in _cache:
        _cache[key] = _build(k_list, nonzero_bias)
    nc = _cache[key]

    common = dict(x=x, xT=np.ascontiguousarray(x.T), **wdict)
    if nonzero_bias:
        common["b1r"] = np.tile(np.asarray(b1, np.float32)[None, :], (128, 1))
        common["b2r"] = np.tile(np.asarray(b2, np.float32)[None, :], (128, 1))
        common["b3r"] = np.tile(np.asarray(b3, np.float32)[None, :], (128, 1))
    in_maps = []
    for c in range(NCORES):
        in_maps.append(dict(common, isrc=isrc[c], idstg=idg[c], idstl=idl[c],
                            S=S[c]))

    if BENCH_REPS:
        import benchlib  # only available in the dev sandbox
        times, outarrs, onames = benchlib.run_timed(nc, in_maps, NCORES,
                                                    reps=BENCH_REPS)
        LAST_BENCH_NS = int(min(times) * 1e9)
        oi = onames.index("out")
        full = np.asarray(outarrs[oi]).reshape(NCORES, 8, OUT)
        return full.reshape(G, OUT)

    res = run_bass_kernel_spmd(nc, in_maps, list(range(NCORES)))
    return np.concatenate([res.results[c]["out"] for c in range(NCORES)],
                          axis=0)


# revision 9
# speedup vs baseline: 1.4342x; 1.4342x over previous
"""BrainGAT (3-layer GAT + mean-pool + projection) on 8 Trainium2 NeuronCores.

Strategy (dst-sharded message passing):
  - Nodes are sharded by destination across the 8 cores (4096 nodes each, whole
    graphs).  Edges (incl. self-loops) are sorted by dst on the host and padded
    into 128-edge tiles per 128-node dst block.
  - Per layer, each core gathers the per-src feature rows (dma_gather), weights
    them with the edge attention p = exp(leaky_relu(als[src]+ald[dst])) and
    scatter-adds into PSUM with a host-built 0/1 selection matrix S via the
    tensor engine:  out[d,:] = S^T @ (p * xw[src]);  denom[d,h] = S^T @ p.
    Softmax max-subtraction is skipped (mathematically identical, logits are
    O(1) so exp never overflows).
  - The aggregation for layer 1 runs in input space (x is only 4 wide) and W1
    is applied after aggregation (linearity of the attention-weighted sum).
  - Between layers each core computes the next layer's gather table rows
    (xw_{l+1}, als_{l+1}, ald_{l+1}) for its own 4096 nodes and the tables are
    AllGathered so every core can gather arbitrary src rows.
  - Final mean-pool is per-graph-local (graphs never cross cores); each core
    emits pooled @ Wp + bp for its 8 graphs, host concatenates.
"""

import sys
import numpy as np

for _p in ("/opt/trn_rl_repo", "/root/.axon_site/_ro/trn_rl_repo"):
    if _p not in sys.path:
        sys.path.insert(0, _p)

import ml_dtypes
import concourse.bass as bass
import concourse.bacc as bacc
import concourse.mybir as mybir
import concourse.tile as tile
from concourse.bass_utils import run_bass_kernel_spmd
from concourse.masks import make_identity

dt = mybir.dt
F32, BF16, I16 = dt.float32, dt.bfloat16, dt.int16
BF16_NP = ml_dtypes.bfloat16

N = 32768
G = 64
IN = 4
H = 8
C = 64
OUT = 384
NEG = 0.2
NCORES = 8
NPC = N // NCORES          # nodes per core
NBLK = NPC // 128          # dst blocks per core
P = 128

BENCH_REPS = 0             # test.py sets >0 to time repeated device calls
LAST_BENCH_NS = None
LAST_TIMES = None

_cache = {}


def _build(k_list, nonzero_bias):
    K_tot = sum(k_list)
    offs = np.concatenate([[0], np.cumsum(k_list)]).astype(int)

    nc = bacc.Bacc("TRN2", target_bir_lowering=False, debug=False,
                   num_devices=NCORES)

    # ---- I/O --------------------------------------------------------------
    x_in = nc.dram_tensor("x", [N, IN], F32, kind="ExternalInput")
    xT_in = nc.dram_tensor("xT", [IN, N], F32, kind="ExternalInput")
    wsd1_in = nc.dram_tensor("wsd1", [IN, 2 * H], F32, kind="ExternalInput")
    w1b_in = nc.dram_tensor("w1b", [H * IN, H * C], BF16, kind="ExternalInput")
    w2c_in = nc.dram_tensor("w2c", [128, 4 * 528], BF16, kind="ExternalInput")
    w3c_in = nc.dram_tensor("w3c", [128, 4 * 66], BF16, kind="ExternalInput")
    wp_in = nc.dram_tensor("wp", [C, OUT], F32, kind="ExternalInput")
    bp_in = nc.dram_tensor("bp", [8, OUT], F32, kind="ExternalInput")
    poh_in = nc.dram_tensor("poh", [128, NBLK * 8], F32, kind="ExternalInput")
    isrc_in = nc.dram_tensor("isrc", [16, K_tot * 8], I16, kind="ExternalInput")
    idstg_in = nc.dram_tensor("idstg", [16, K_tot * 8], I16, kind="ExternalInput")
    idstl_in = nc.dram_tensor("idstl", [16, K_tot * 8], I16, kind="ExternalInput")
    S_in = nc.dram_tensor("S", [128, K_tot * 128], BF16, kind="ExternalInput")
    out_ext = nc.dram_tensor("out", [8, OUT], F32, kind="ExternalOutput")
    if nonzero_bias:
        b1_in = nc.dram_tensor("b1r", [128, H * C], F32, kind="ExternalInput")
        b2_in = nc.dram_tensor("b2r", [128, H * C], F32, kind="ExternalInput")
        b3_in = nc.dram_tensor("b3r", [128, C], F32, kind="ExternalInput")

    # ---- internal DRAM tables --------------------------------------------
    # aux1: [x(4) | als1(8) | ald1(8) | pad] f32, 256B rows, replicated per core
    aux1 = nc.dram_tensor("aux1", [N, 64], F32)
    # main2: [xw2 bf16 512 (256w) | als2 f32 8 | pad] 1280B rows
    main2_own = nc.dram_tensor("main2_own", [NPC, 320], F32)
    main2 = nc.dram_tensor("main2", [N, 320], F32, addr_space="Shared")
    # auxd2: [ald2 f32 8 | pad] 256B rows, core-local (indexed by local dst)
    auxd2 = nc.dram_tensor("auxd2", [NPC, 64], F32)
    # main3: [xw3 bf16 64 (32w) | als3 f32 | ald3 f32 | pad] 256B rows
    main3_own = nc.dram_tensor("main3_own", [NPC, 64], F32)
    main3 = nc.dram_tensor("main3", [N, 64], F32, addr_space="Shared")

    RG = [list(range(NCORES))]

    with tile.TileContext(nc) as tc:
        with tc.tile_pool(name="const", bufs=1) as cst, \
             tc.tile_pool(name="gbuf", bufs=2) as sbg, \
             tc.tile_pool(name="small", bufs=2) as sbm, \
             tc.tile_pool(name="psA", bufs=2, space="PSUM") as psA, \
             tc.tile_pool(name="psD", bufs=1, space="PSUM") as psD, \
             tc.tile_pool(name="psT", bufs=1, space="PSUM") as psT, \
             tc.tile_pool(name="psH", bufs=1, space="PSUM") as psH, \
             tc.tile_pool(name="psX", bufs=1, space="PSUM") as psX, \
             tc.tile_pool(name="psS", bufs=1, space="PSUM") as psS, \
             tc.tile_pool(name="psP", bufs=1, space="PSUM") as psP:

            # ---- constants into SBUF -------------------------------------
            ident = cst.tile([128, 128], F32)
            make_identity(nc, ident[:])
            identb = cst.tile([128, 128], BF16)
            nc.vector.tensor_copy(out=identb[:], in_=ident[:])
            wsd1 = cst.tile([IN, 2 * H], F32)
            nc.sync.dma_start(out=wsd1[:], in_=wsd1_in[:, :])
            w1b = cst.tile([H * IN, H * C], BF16)
            nc.sync.dma_start(out=w1b[:], in_=w1b_in[:, :])
            w2c = cst.tile([128, 4 * 528], BF16)
            nc.sync.dma_start(out=w2c[:], in_=w2c_in[:, :])
            w3c = cst.tile([128, 4 * 66], BF16)
            nc.sync.dma_start(out=w3c[:], in_=w3c_in[:, :])
            wp = cst.tile([C, OUT], F32)
            nc.sync.dma_start(out=wp[:], in_=wp_in[:, :])
            bp = cst.tile([8, OUT], F32)
            nc.sync.dma_start(out=bp[:], in_=bp_in[:, :])
            poh = cst.tile([128, NBLK * 8], F32)
            nc.sync.dma_start(out=poh[:], in_=poh_in[:, :])
            if nonzero_bias:
                b1r = cst.tile([128, H * C], F32)
                nc.sync.dma_start(out=b1r[:], in_=b1_in[:, :])
                b2r = cst.tile([128, H * C], F32)
                nc.sync.dma_start(out=b2r[:], in_=b2_in[:, :])
                b3r = cst.tile([128, C], F32)
                nc.sync.dma_start(out=b3r[:], in_=b3_in[:, :])

            isrc = cst.tile([128, K_tot * 8], I16)
            idstg = cst.tile([128, K_tot * 8], I16)
            idstl = cst.tile([128, K_tot * 8], I16)
            for g in range(8):
                nc.sync.dma_start(out=isrc[g * 16:(g + 1) * 16, :], in_=isrc_in[:, :])
                nc.sync.dma_start(out=idstg[g * 16:(g + 1) * 16, :], in_=idstg_in[:, :])
                nc.sync.dma_start(out=idstl[g * 16:(g + 1) * 16, :], in_=idstl_in[:, :])

            # ---- stage 0: aux1 = [x | als1 | ald1] for ALL N (replicated) -
            nc.sync.dma_start(out=aux1[:, 0:IN], in_=x_in[:, :])
            for t in range(N // 128):
                if t % 16 == 0:
                    xtc = sbm.tile([IN, 16 * 128], F32, tag="xtc", bufs=2)
                    nc.sync.dma_start(out=xtc[:], in_=xT_in[:, t * 128:(t + 16) * 128])
                ps0 = psA.tile([128, 2 * H], F32, tag="agg")
                nc.tensor.matmul(out=ps0[:], lhsT=xtc[:, (t % 16) * 128:(t % 16 + 1) * 128],
                                 rhs=wsd1[:], start=True, stop=True)
                st0 = sbm.tile([128, 2 * H], F32, tag="st0", bufs=3)
                nc.vector.tensor_copy(out=st0[:], in_=ps0[:])
                nc.sync.dma_start(out=aux1[t * 128:(t + 1) * 128, IN:IN + 2 * H], in_=st0[:])

            # ---- layer template ------------------------------------------
            def layer(l):
                if l == 1:
                    main_tbl, Rw = aux1, 64
                    dst_tbl, Dw, idx_d = aux1, 64, idstg
                    a_s0, d_s0, Hl, Wl = IN, IN + H, H, H * IN
                elif l == 2:
                    main_tbl, Rw = main2, 320
                    dst_tbl, Dw, idx_d = auxd2, 64, idstl
                    a_s0, d_s0, Hl, Wl = 256, 0, H, H * C
                else:
                    main_tbl, Rw = main3, 64
                    dst_tbl, Dw, idx_d = main3, 64, idstg
                    a_s0, d_s0, Hl, Wl = 32, 33, 1, C

                for b in range(NBLK):
                    k = k_list[b]
                    off = int(offs[b])
                    nidx = k * 128
                    Gm = sbg.tile([128, k * Rw], F32, tag="gmain")
                    nc.gpsimd.dma_gather(
                        out_ap=Gm[:].rearrange("p (k r) -> p k r", r=Rw),
                        in_ap=main_tbl[:, :],
                        idxs_ap=isrc[:, off * 8:(off + k) * 8],
                        num_idxs=nidx, num_idxs_reg=nidx, elem_size=Rw,
                        single_packet=False)
                    Gd = sbg.tile([128, k * Dw], F32, tag="gdst")
                    nc.gpsimd.dma_gather(
                        out_ap=Gd[:].rearrange("p (k r) -> p k r", r=Dw),
                        in_ap=dst_tbl[:, :],
                        idxs_ap=idx_d[:, off * 8:(off + k) * 8],
                        num_idxs=nidx, num_idxs_reg=nidx, elem_size=Dw,
                        single_packet=False)
                    S = sbg.tile([128, k * 128], BF16, tag="S")
                    nc.sync.dma_start(out=S[:], in_=S_in[:, off * 128:(off + k) * 128])

                    # edge logits -> p
                    ppre = sbm.tile([128, k * Hl], F32, tag="ppre")
                    nc.vector.tensor_tensor(
                        out=ppre[:].rearrange("p (k h) -> p k h", h=Hl),
                        in0=Gm[:].rearrange("p (k r) -> p k r", r=Rw)[:, :, a_s0:a_s0 + Hl],
                        in1=Gd[:].rearrange("p (k r) -> p k r", r=Dw)[:, :, d_s0:d_s0 + Hl],
                        op=mybir.AluOpType.add)
                    psc = sbm.tile([128, k * Hl], F32, tag="psc")
                    nc.vector.tensor_scalar(out=psc[:], in0=ppre[:], scalar1=NEG,
                                            scalar2=None, op0=mybir.AluOpType.mult)
                    plog = sbm.tile([128, k * Hl], F32, tag="plog")
                    nc.vector.tensor_tensor(out=plog[:], in0=ppre[:], in1=psc[:],
                                            op=mybir.AluOpType.max)
                    pf = sbm.tile([128, k * Hl], F32, tag="pf")
                    nc.scalar.activation(out=pf[:], in_=plog[:],
                                         func=mybir.ActivationFunctionType.Exp)
                    pb = sbm.tile([128, k * Hl], BF16, tag="pb")
                    nc.vector.tensor_copy(out=pb[:], in_=pf[:])

                    agg = psA.tile([128, Wl], F32, tag="agg")
                    den = psD.tile([128, Hl], F32, tag="den")
                    for j in range(k):
                        Gw = sbm.tile([128, Wl], BF16, tag="gw", bufs=3)
                        if l == 1:
                            nc.vector.tensor_tensor(
                                out=Gw[:].rearrange("p (h c) -> p h c", c=IN),
                                in0=Gm[:, j * Rw:j * Rw + IN].unsqueeze(1).to_broadcast([128, H, IN]),
                                in1=pf[:, j * H:(j + 1) * H].unsqueeze(2).to_broadcast([128, H, IN]),
                                op=mybir.AluOpType.mult)
                        elif l == 2:
                            nc.vector.tensor_tensor(
                                out=Gw[:].rearrange("p (h c) -> p h c", c=C),
                                in0=Gm[:, j * Rw:j * Rw + 256].bitcast(BF16).rearrange(
                                    "p (h c) -> p h c", c=C),
                                in1=pf[:, j * H:(j + 1) * H].unsqueeze(2).to_broadcast([128, H, C]),
                                op=mybir.AluOpType.mult)
                        else:
                            nc.vector.tensor_tensor(
                                out=Gw[:],
                                in0=Gm[:, j * Rw:j * Rw + 32].bitcast(BF16),
                                in1=pf[:, j:j + 1].to_broadcast([128, C]),
                                op=mybir.AluOpType.mult)
                        Sj = S[:, j * 128:(j + 1) * 128]
                        nc.tensor.matmul(out=agg[:], lhsT=Sj, rhs=Gw[:],
                                         start=(j == 0), stop=(j == k - 1))
                        nc.tensor.matmul(out=den[:], lhsT=Sj,
                                         rhs=pb[:, j * Hl:(j + 1) * Hl],
                                         start=(j == 0), stop=(j == k - 1))

                    rden = sbm.tile([128, Hl], F32, tag="rden")
                    nc.vector.reciprocal(out=rden[:], in_=den[:])

                    if l == 1:
                        A1 = sbm.tile([128, Wl], BF16, tag="a1")
                        nc.vector.tensor_tensor(
                            out=A1[:].rearrange("p (h c) -> p h c", c=IN),
                            in0=agg[:].rearrange("p (h c) -> p h c", c=IN),
                            in1=rden[:].unsqueeze(2).to_broadcast([128, H, IN]),
                            op=mybir.AluOpType.mult)
                        pT = psT.tile([Wl, 128], BF16, tag="tr")
                        nc.tensor.transpose(out=pT[:], in_=A1[:], identity=identb[:])
                        A1T = sbm.tile([Wl, 128], BF16, tag="a1t")
                        nc.vector.tensor_copy(out=A1T[:], in_=pT[:])
                        hps = psH.tile([128, H * C], F32, tag="h")
                        nc.tensor.matmul(out=hps[:], lhsT=A1T[:], rhs=w1b[:],
                                         start=True, stop=True)
                        u_src = hps
                    else:
                        u = sbm.tile([128, Wl], F32, tag="u")
                        if l == 2:
                            nc.vector.tensor_tensor(
                                out=u[:].rearrange("p (h c) -> p h c", c=C),
                                in0=agg[:].rearrange("p (h c) -> p h c", c=C),
                                in1=rden[:].unsqueeze(2).to_broadcast([128, H, C]),
                                op=mybir.AluOpType.mult)
                        else:
                            nc.vector.tensor_tensor(
                                out=u[:], in0=agg[:],
                                in1=rden[:].to_broadcast([128, C]),
                                op=mybir.AluOpType.mult)
                        u_src = u

                    if nonzero_bias:
                        br = b1r if l == 1 else (b2r if l == 2 else b3r)
                        ub = sbm.tile([128, Wl], F32, tag="ub")
                        nc.vector.tensor_tensor(out=ub[:], in0=u_src[:],
                                                in1=br[:, 0:Wl],
                                                op=mybir.AluOpType.add)
                        u_src = ub

                    HWl = H * C if l == 1 else Wl
                    m_ = sbm.tile([128, HWl], F32, tag="m")
                    nc.vector.tensor_scalar(out=m_[:], in0=u_src[:], scalar1=0.0,
                                            scalar2=None, op0=mybir.AluOpType.min)
                    e_ = sbm.tile([128, HWl], F32, tag="e")
                    nc.scalar.activation(out=e_[:], in_=m_[:],
                                         func=mybir.ActivationFunctionType.Exp)
                    r_ = sbm.tile([128, HWl], F32, tag="r")
                    nc.vector.tensor_scalar(out=r_[:], in0=u_src[:], scalar1=0.0,
                                            scalar2=-1.0, op0=mybir.AluOpType.max,
                                            op1=mybir.AluOpType.add)
                    hdt = F32 if l == 3 else BF16
                    h = sbm.tile([128, HWl], hdt, tag="hout")
                    nc.vector.tensor_tensor(out=h[:], in0=r_[:], in1=e_[:],
                                            op=mybir.AluOpType.add)

                    if l == 1 or l == 2:
                        hTs = []
                        for ch in range(4):
                            tps = psT.tile([128, 128], BF16, tag="tr")
                            nc.tensor.transpose(out=tps[:],
                                                in_=h[:, ch * 128:(ch + 1) * 128],
                                                identity=identb[:])
                            hT = sbm.tile([128, 128], BF16, tag=f"ht{ch}")
                            nc.vector.tensor_copy(out=hT[:], in_=tps[:])
                            hTs.append(hT)
                        if l == 1:
                            xw = psX.tile([128, 512], F32, tag="xw")
                            als = psS.tile([128, 16], F32, tag="als")
                            for ch in range(4):
                                nc.tensor.matmul(out=xw[:], lhsT=hTs[ch][:],
                                                 rhs=w2c[:, ch * 528:ch * 528 + 512],
                                                 start=(ch == 0), stop=(ch == 3))
                            for ch in range(4):
                                nc.tensor.matmul(out=als[:], lhsT=hTs[ch][:],
                                                 rhs=w2c[:, ch * 528 + 512:(ch + 1) * 528],
                                                 start=(ch == 0), stop=(ch == 3))
                            stg = sbm.tile([128, 320], F32, tag="stg")
                            nc.vector.tensor_copy(out=stg[:, 0:256].bitcast(BF16), in_=xw[:])
                            nc.vector.tensor_copy(out=stg[:, 256:264], in_=als[:, 0:8])
                            stga = sbm.tile([128, 64], F32, tag="stga")
                            nc.vector.tensor_copy(out=stga[:, 0:8], in_=als[:, 8:16])
                            nc.sync.dma_start(out=main2_own[b * 128:(b + 1) * 128, :], in_=stg[:])
                            nc.sync.dma_start(out=auxd2[b * 128:(b + 1) * 128, :], in_=stga[:])
                        else:
                            x3 = psX.tile([128, 66], F32, tag="xw")
                            for ch in range(4):
                                nc.tensor.matmul(out=x3[:], lhsT=hTs[ch][:],
                                                 rhs=w3c[:, ch * 66:(ch + 1) * 66],
                                                 start=(ch == 0), stop=(ch == 3))
                            stg3 = sbm.tile([128, 64], F32, tag="stg")
                            nc.vector.tensor_copy(out=stg3[:, 0:32].bitcast(BF16), in_=x3[:, 0:64])
                            nc.vector.tensor_copy(out=stg3[:, 32:34], in_=x3[:, 64:66])
                            nc.sync.dma_start(out=main3_own[b * 128:(b + 1) * 128, :], in_=stg3[:])
                    else:
                        nc.tensor.matmul(out=pool_ps[:],
                                         lhsT=poh[:, b * 8:(b + 1) * 8],
                                         rhs=h[:], start=(b == 0), stop=(b == NBLK - 1))

            layer(1)
            nc.gpsimd.collective_compute(
                "AllGather", mybir.AluOpType.bypass, replica_groups=RG,
                ins=[main2_own[:, :]], outs=[main2[:, :]])
            layer(2)
            nc.gpsimd.collective_compute(
                "AllGather", mybir.AluOpType.bypass, replica_groups=RG,
                ins=[main3_own[:, :]], outs=[main3[:, :]])
            pool_ps = psP.tile([8, C], F32, tag="pool")
            layer(3)

            pooled = sbm.tile([8, C], F32, tag="pooled")
            nc.vector.tensor_scalar(out=pooled[:], in0=pool_ps[:],
                                    scalar1=1.0 / 512.0, scalar2=None,
                                    op0=mybir.AluOpType.mult)
            pTps = psT.tile([C, 8], F32, tag="tr")
            nc.tensor.transpose(out=pTps[:], in_=pooled[:], identity=ident[0:8, 0:8])
            pT = sbm.tile([C, 8], F32, tag="pT")
            nc.vector.tensor_copy(out=pT[:], in_=pTps[:])
            ops = psX.tile([8, OUT], F32, tag="xw")
            nc.tensor.matmul(out=ops[:], lhsT=pT[:], rhs=wp[:], start=True, stop=True)
            ores = sbm.tile([8, OUT], F32, tag="ores")
            nc.vector.tensor_tensor(out=ores[:], in0=ops[:], in1=bp[:],
                                    op=mybir.AluOpType.add)
            nc.sync.dma_start(out=out_ext[:, :], in_=ores[:])

    nc.compile()
    return nc


def _wrap16(a):
    # flat index position i -> [i % 16, i // 16] int16
    n = a.shape[-1]
    return np.ascontiguousarray(a.reshape(*a.shape[:-1], n // 16, 16).swapaxes(-1, -2))


def _prep_edges(edge_index):
    src = np.concatenate([np.asarray(edge_index[0], np.int64), np.arange(N, dtype=np.int64)])
    dst = np.concatenate([np.asarray(edge_index[1], np.int64), np.arange(N, dtype=np.int64)])
    order = np.argsort(dst, kind="stable")
    src_s, dst_s = src[order], dst[order]
    blk = dst_s // 128
    counts = np.bincount(blk, minlength=NCORES * NBLK)
    cgrid = counts.reshape(NCORES, NBLK)
    k_list = [max(1, int(np.ceil(cgrid[:, b].max() / 128.0))) for b in range(NBLK)]
    offs = np.concatenate([[0], np.cumsum(k_list)]).astype(int)
    K_tot = int(offs[-1])
    bstart = np.concatenate([[0], np.cumsum(counts)]).astype(int)

    isrc = np.zeros((NCORES, K_tot * 128), np.int16)
    idg = np.zeros((NCORES, K_tot * 128), np.int16)
    idl = np.zeros((NCORES, K_tot * 128), np.int16)
    S = np.zeros((NCORES, 128, K_tot * 128), BF16_NP)
    for c in range(NCORES):
        for b in range(NBLK):
            g = c * NBLK + b
            s0, s1 = bstart[g], bstart[g + 1]
            ne = s1 - s0
            off = int(offs[b])
            pos = slice(off * 128, off * 128 + ne)
            isrc[c, pos] = src_s[s0:s1]
            idg[c, pos] = dst_s[s0:s1]
            idl[c, pos] = dst_s[s0:s1] - c * NPC
            i = np.arange(ne)
            dloc = dst_s[s0:s1] - g * 128
            S[c, i % 128, (off + i // 128) * 128 + dloc] = 1.0
            # padding edges: idx stay 0 (valid row), S rows stay all-zero
    return k_list, _wrap16(isrc), _wrap16(idg), _wrap16(idl), S


def _prep_weights(W1, a1s, a1d, W2, a2s, a2d, W3, a3s, a3d, Wp, bp):
    f8 = np.float64

    def fuse(W, a):  # [D, Hl*C] x [Hl, C] -> [D, Hl]
        Hl = a.shape[0]
        Wr = W.astype(f8).reshape(W.shape[0], Hl, C)
        return np.einsum("ihc,hc->ih", Wr, a.astype(f8)).astype(np.float32)

    wsd1 = np.concatenate([fuse(W1, a1s), fuse(W1, a1d)], axis=1)  # [4, 16]
    w1b = np.zeros((H * IN, H * C), np.float32)
    for h in range(H):
        w1b[h * IN:(h + 1) * IN, h * C:(h + 1) * C] = W1[:, h * C:(h + 1) * C]
    w2cat = np.concatenate([W2, fuse(W2, a2s), fuse(W2, a2d)], axis=1)  # [512, 528]
    w3cat = np.concatenate([W3, fuse(W3, a3s), fuse(W3, a3d)], axis=1)  # [512, 66]

    def chunks(Wc):  # [512, M] -> [128, 4*M]
        return np.concatenate([Wc[ch * 128:(ch + 1) * 128, :] for ch in range(4)],
                              axis=1)

    poh = np.zeros((128, NBLK * 8), np.float32)
    for b in range(NBLK):
        poh[:, b * 8 + b // 4] = 1.0
    return dict(
        wsd1=wsd1.astype(np.float32),
        w1b=w1b.astype(BF16_NP),
        w2c=chunks(w2cat).astype(BF16_NP),
        w3c=chunks(w3cat).astype(BF16_NP),
        wp=np.asarray(Wp, np.float32),
        bp=np.tile(np.asarray(bp, np.float32)[None, :], (8, 1)),
        poh=poh,
    )


def kernel(x, edge_index, edge_attr, batch,
           W1, a1s, a1d, b1, W2, a2s, a2d, b2, W3, a3s, a3d, b3, Wp, bp):
    global LAST_BENCH_NS
    x = np.asarray(x, np.float32)
    k_list, isrc, idg, idl, S = _prep_edges(np.asarray(edge_index))
    wdict = _prep_weights(np.asarray(W1), np.asarray(a1s), np.asarray(a1d),
                          np.asarray(W2), np.asarray(a2s), np.asarray(a2d),
                          np.asarray(W3), np.asarray(a3s), np.asarray(a3d),
                          np.asarray(Wp), np.asarray(bp))
    nonzero_bias = any(np.any(np.asarray(b) != 0) for b in (b1, b2, b3))

    key = (tuple(k_list), nonzero_bias)
    if key not in _cache:
        _cache[key] = _build(k_list, nonzero_bias)
    nc = _cache[key]

    common = dict(x=x, xT=np.ascontiguousarray(x.T), **wdict)
    if nonzero_bias:
        common["b1r"] = np.tile(np.asarray(b1, np.float32)[None, :], (128, 1))
        common["b2r"] = np.tile(np.asarray(b2, np.float32)[None, :], (128, 1))
        common["b3r"] = np.tile(np.asarray(b3, np.float32)[None, :], (128, 1))
    in_maps = []
    for c in range(NCORES):
        in_maps.append(dict(common, isrc=isrc[c], idstg=idg[c], idstl=idl[c],
                            S=S[c]))

    if BENCH_REPS:
        import benchlib  # only available in the dev sandbox
        times, outarrs, onames = benchlib.run_timed(nc, in_maps, NCORES,
                                                    reps=BENCH_REPS)
        global LAST_TIMES
        LAST_TIMES = sorted(t * 1e3 for t in times)
        LAST_BENCH_NS = int(min(times) * 1e9)
        oi = onames.index("out")
        full = np.asarray(outarrs[oi]).reshape(NCORES, 8, OUT)
        return full.reshape(G, OUT)

    res = run_bass_kernel_spmd(nc, in_maps, list(range(NCORES)))
    return np.concatenate([res.results[c]["out"] for c in range(NCORES)],
                          axis=0)
